# revision 8
# baseline (speedup 1.0000x reference)
"""GQA causal attention (S=2048, H=32, KVH=8, D=128) on 8 TRN2 NeuronCores.

Sharding: tensor-parallel over heads. Core i computes query heads
[4i, 4i+4) against KV head i (GQA group size 32/8 = 4). No collectives:
the host slices the inputs per core and concatenates the outputs.

Per-core algorithm (seq=2048, d=128, 4 q-heads, 1 kv-head, causal):
  - K^T and per-head Q^T staged in SBUF as [d=128, seq] bf16
    (PE transposes via identity matmul; fp32 DMA-transpose is unsupported).
  - V staged naturally as [128, 16, 129] bf16 tiles with a ones column
    appended, so the PV matmul also produces the softmax denominator.
  - For each head, for each key-tile kt (128 keys):
      S^T[kt]  = (K^T tile).T @ Q^T          -> PSUM [128, qspan] fp32,
                 only the causal span q >= kt*128 is computed
      P^T[kt]  = exp(SCALE * S^T[kt])        -> SBUF bf16 (one wide ACTIVATE;
                 scores are O(1) so no max-subtraction is needed in fp32/bf16)
      the diagonal 128-column block is masked with gpsimd.affine_select
  - For each query-tile qt: acc[qt] = sum_kt (P^T tile).T @ [V | 1]
      accumulated in PSUM over kt; column 128 is the denominator.
      DVE reciprocal + tensor_scalar_mul normalizes; result DMAs out in
      the natural [seq, d] layout.
"""

import numpy as np

SEQ = 2048
D = 128
QH = 4  # query heads per core
N_CORES = 8
SCALE = 0.08838834764831845  # 1/sqrt(128)
NT = SEQ // 128  # 16 tiles of 128 along seq

_NC = None


def _emit(ctx, tc, q, k, v, out):
    import concourse.mybir as mybir
    from concourse import masks

    nc = tc.nc
    f32 = mybir.dt.float32
    bf16 = mybir.dt.bfloat16
    Exp = mybir.ActivationFunctionType.Exp

    # Every DMA destination gets a dedicated (never-recycled) buffer: a
    # reused slot would force >1 semaphore wait on the HWDGE DMA, which
    # walrus rejects ("Too many sync wait commands").
    singles = ctx.enter_context(tc.tile_pool(name="singles", bufs=1))
    qpool = ctx.enter_context(tc.tile_pool(name="qpool", bufs=2))
    ppool = ctx.enter_context(tc.tile_pool(name="ppool", bufs=2))
    opool = ctx.enter_context(tc.tile_pool(name="opool", bufs=3))
    psum_s = ctx.enter_context(tc.tile_pool(name="psum_s", bufs=2, space="PSUM"))
    psum_o = ctx.enter_context(tc.tile_pool(name="psum_o", bufs=2, space="PSUM"))
    psum_t = ctx.enter_context(tc.tile_pool(name="psum_t", bufs=2, space="PSUM"))

    ident = singles.tile([128, 128], bf16)
    masks.make_identity(nc, ident[:])

    # ---- K: load natural fp32, cast bf16, PE-transpose into kT [d, seq]
    kT = singles.tile([128, SEQ], bf16)
    knat = singles.tile([128, NT, 128], f32, tag="knat")
    nc.sync.dma_start(out=knat[:], in_=k.rearrange("(t p) d -> p t d", p=128))
    knat_bf = singles.tile([128, NT, 128], bf16, tag="knat_bf")
    nc.vector.tensor_copy(knat_bf[:], knat[:])
    for t in range(NT):
        pst = psum_t.tile([128, 128], bf16, tag="tp")
        nc.tensor.transpose(pst[:], knat_bf[:, t, :], ident[:])
        nc.vector.tensor_copy(kT[:, t * 128:(t + 1) * 128], pst[:])

    # ---- V: natural [128, t, d] bf16 + ones column for the denominator
    vp = singles.tile([128, NT, D + 1], bf16)
    vnat = singles.tile([128, NT, 128], f32, tag="vnat")
    nc.sync.dma_start(out=vnat[:], in_=v.rearrange("(t p) d -> p t d", p=128))
    nc.vector.tensor_copy(vp[:, :, 0:D], vnat[:])
    nc.vector.memset(vp[:, :, D:D + 1], 1.0)

    def emit_pv(h, qt, pT):
        """O[qt] = sum_k2 pT[k2][:, qt-slice].T @ [V|1], then normalize."""
        ops = psum_o.tile([128, D + 1], f32, tag="o")
        for k2 in range(qt + 1):
            nc.tensor.matmul(
                ops[:],
                lhsT=pT[k2][:, (qt - k2) * 128:(qt - k2) * 128 + 128],
                rhs=vp[:, k2, :],
                start=(k2 == 0),
                stop=(k2 == qt),
            )
        rec = opool.tile([128, 1], f32, tag="rec")
        nc.vector.reciprocal(rec[:], ops[:, D:D + 1])
        osb = opool.tile([128, D], f32, tag="osb")
        nc.vector.tensor_scalar_mul(osb[:], ops[:, 0:D], rec[:])
        nc.sync.dma_start(
            out=out[qt * 128:(qt + 1) * 128, h * D:(h + 1) * D], in_=osb[:]
        )

    for h in range(QH):
        # ---- Q prep for head h: load, cast, PE-transpose into qT [d, seq]
        qnat = singles.tile([128, NT, 128], f32, tag=f"qnat{h}")
        nc.sync.dma_start(
            out=qnat[:],
            in_=q[:, h * D:(h + 1) * D].rearrange("(t p) d -> p t d", p=128),
        )
        qnat_bf = singles.tile([128, NT, 128], bf16, tag=f"qnat_bf{h}")
        nc.vector.tensor_copy(qnat_bf[:], qnat[:])
        qT = qpool.tile([128, SEQ], bf16, tag="qT")
        for t in range(NT):
            pst = psum_t.tile([128, 128], bf16, tag="tp")
            nc.tensor.transpose(pst[:], qnat_bf[:, t, :], ident[:])
            nc.vector.tensor_copy(qT[:, t * 128:(t + 1) * 128], pst[:])

        pT = []
        for kt in range(NT):
            k0 = kt * 128
            pT_kt = ppool.tile([128, SEQ - k0], bf16, tag=f"pT{kt}")
            # S^T and exp, in 1024-wide PSUM chunks (2 banks each)
            for qc in range(k0 // 1024, SEQ // 1024):
                c0 = qc * 1024
                sp = psum_s.tile([128, 1024], f32, tag="s")
                for qb in range(max(k0 // 512, c0 // 512), c0 // 512 + 2):
                    nc.tensor.matmul(
                        sp[:, qb * 512 - c0:qb * 512 - c0 + 512],
                        lhsT=kT[:, k0:k0 + 128],
                        rhs=qT[:, qb * 512:(qb + 1) * 512],
                        start=True,
                        stop=True,
                    )
                q0 = max(k0, c0)
                nc.scalar.activation(
                    pT_kt[:, q0 - k0:c0 + 1024 - k0],
                    sp[:, q0 - c0:1024],
                    Exp,
                    scale=SCALE,
                )
            # causal mask on the diagonal 128-col block: keep where q >= k
            nc.gpsimd.affine_select(
                out=pT_kt[:, 0:128],
                in_=pT_kt[:, 0:128],
                compare_op=mybir.AluOpType.is_ge,
                fill=0.0,
                base=0,
                pattern=[[1, 128]],
                channel_multiplier=-1,
            )
            pT.append(pT_kt)
            # software-pipeline PV one key-tile behind QK/exp
            if kt >= 1:
                emit_pv(h, kt - 1, pT)
        emit_pv(h, NT - 1, pT)


def _build():
    import concourse.mybir as mybir
    import concourse.tile as tile
    from concourse import bacc
    from contextlib import ExitStack

    nc = bacc.Bacc()
    q = nc.declare_dram_parameter("q", [SEQ, QH * D], mybir.dt.float32, isOutput=False)
    k = nc.declare_dram_parameter("k", [SEQ, D], mybir.dt.float32, isOutput=False)
    v = nc.declare_dram_parameter("v", [SEQ, D], mybir.dt.float32, isOutput=False)
    out = nc.declare_dram_parameter("out", [SEQ, QH * D], mybir.dt.float32, isOutput=True)

    with tile.TileContext(nc) as tc:
        with ExitStack() as ctx:
            _emit(ctx, tc, q[:], k[:], v[:], out[:])
    nc.compile()
    return nc


def _get_nc():
    global _NC
    if _NC is None:
        _NC = _build()
    return _NC


def _ensure_ntff_hook():
    """The agent image's antenv lacks axon_hooks; shim it so trace=True works."""
    import sys
    import types

    if "antenv.axon_hooks" in sys.modules:
        return
    try:
        import antenv
        from trn_agent_boot.trn_boot import _ntff_profile_via_ctypes
    except ImportError:
        return
    mod = types.ModuleType("antenv.axon_hooks")
    hook = [None]
    mod.set_axon_ntff_profile_hook = lambda h: hook.__setitem__(0, h)
    mod.get_axon_ntff_profile_hook = lambda: hook[0]
    sys.modules["antenv.axon_hooks"] = mod
    antenv.axon_hooks = mod
    mod.set_axon_ntff_profile_hook(_ntff_profile_via_ctypes("/opt/axon/libaxon_pjrt.so"))


def _run(q, k, v, trace=False):
    from concourse.bass_utils import run_bass_kernel_spmd

    if trace:
        _ensure_ntff_hook()
    nc = _get_nc()
    in_maps = []
    for i in range(N_CORES):
        in_maps.append(
            {
                "q": np.ascontiguousarray(q[:, i * QH * D:(i + 1) * QH * D]).astype(np.float32, copy=False),
                "k": np.ascontiguousarray(k[:, i * D:(i + 1) * D]).astype(np.float32, copy=False),
                "v": np.ascontiguousarray(v[:, i * D:(i + 1) * D]).astype(np.float32, copy=False),
            }
        )
    res = run_bass_kernel_spmd(nc, in_maps, core_ids=list(range(N_CORES)), trace=trace)
    full = np.concatenate([res.results[i]["out"] for i in range(N_CORES)], axis=1)
    return full.astype(np.float32, copy=False), res


def kernel(q, k, v):
    out, _ = _run(q, k, v, trace=False)
    return out
